# revision 9
# baseline (speedup 1.0000x reference)
"""Trainium2 Bass kernel for nn_ConvFilter (dense_cnn).

Math: tau = sigmoid((x[:,-1,:] @ W.T + b)/10 - 3) in (0, 0.08];
norm = 1/sum_i tau^i = (1-tau)/(1-tau^128) == 1-tau in fp32 (tau^128
underflows); out[b,t,f] = norm * sum_{j=0}^{127} tau^j x[b, t+127-j, f]
== norm * y[b, t+127, f] where y is the infinite exponential filter
y[s] = tau*y[s-1] + x[s] (extra terms carry tau^{>=128} == 0 in fp32).
y is computed with the DVE tensor_tensor_scan recurrence.

Sharding: data-parallel over batch B=32 across 8 cores (4 samples each);
W, b replicated.  Per core: PE transposes x[l,f] tiles into [f,l] PSUM
banks, DVE scans them (chained, state carried across 512-wide chunks),
PE applies norm + transposes back via diag(norm) matmuls, ACT copies
PSUM->SBUF staging, DMA out.

v2: software-pipelined emission — the output stage of sample bi-1
(PE diag-matmuls + ACT copies + DMA out) is interleaved chunk-by-chunk
with the input stage of sample bi (DMA in + PE transposes + DVE scans)
so every engine's in-order queue always has ready work.  y and the
diag(norm) matrices are fp16 (scan state stays fp32 internally), which
runs the 256 output matmuls/core at 1 cycle/row instead of fp32's 4.
"""
import os

import numpy as np

B, L, F, K = 32, 2048, 512, 128
LOUT = L - K + 1  # 1921
NCORES = 8
BC = B // NCORES  # 4 batch samples per core
NF = F // 128     # 4 feature chunks
NL4 = 4           # 1 MiB input chunks per sample (each 512 time steps)

_CACHE = {}


def _build():
    import concourse.bacc as bacc
    import concourse.tile as tile
    from concourse import masks, mybir

    F32 = mybir.dt.float32
    F16 = mybir.dt.float16
    MULT = mybir.AluOpType.mult
    ADD = mybir.AluOpType.add
    SIG = mybir.ActivationFunctionType.Sigmoid

    y_dt = F16 if os.environ.get("Y_DT", "f16") == "f16" else F32

    nc = bacc.Bacc("TRN2", target_bir_lowering=False, debug=False,
                   num_devices=NCORES)
    x_in = nc.dram_tensor("x", [BC, L, F], F32, kind="ExternalInput")
    w_in = nc.dram_tensor("W", [F, F], F32, kind="ExternalInput")
    b_in = nc.dram_tensor("b", [F], F32, kind="ExternalInput")
    out_t = nc.dram_tensor("out", [BC, LOUT, F], F32, kind="ExternalOutput")

    with tile.TileContext(nc) as tc:
        with (
            tc.tile_pool(name="const", bufs=1) as const,
            tc.tile_pool(name="xn", bufs=5) as xn_pool,
            tc.tile_pool(name="taub", bufs=8) as taub_pool,
            tc.tile_pool(name="y", bufs=8) as y_pool,
            tc.tile_pool(name="st", bufs=3) as st_pool,
            tc.tile_pool(name="px", bufs=4, space="PSUM") as px_pool,
            tc.tile_pool(name="po", bufs=3, space="PSUM") as po_pool,
        ):
            ident = const.tile([128, 128], F32)
            masks.make_identity(nc, ident[:])
            ones = const.tile([128, 512], F32)
            nc.gpsimd.memset(ones[:], 1.0)

            # ---- phase 0: tau / norm / diag(norm) ----
            # tau-phase loads go on the ACT HWDGE ring so the Sync ring is
            # free for the (dependency-less) x input stream from t=0.
            featn = const.tile([128, 512], F32)  # rows 0-3 feat, row 4 = b
            nc.gpsimd.memset(featn[:], 0.0)
            nc.scalar.dma_start(featn[0:BC, :], x_in[:, L - 1, :])
            nc.scalar.dma_start(featn[4:5, :], b_in[None, :])
            wn = const.tile([128, NF * 512], F32)
            wn3 = wn[:].rearrange("p (c f) -> p c f", f=512)
            nc.scalar.dma_start(wn3, w_in[:, :].rearrange("(c p) f -> p c f", p=128))

            WT = const.tile([128, NF * 512], F32)   # [fi, (c_fi, fo)]
            WT3 = WT[:].rearrange("p (c f) -> p c f", f=512)
            for cfi in range(NF):
                pt = po_pool.tile([128, 512], F32, tag="po")
                for cfo in range(NF):
                    nc.tensor.transpose(
                        pt[:, 128 * cfo:128 * (cfo + 1)],
                        wn3[:, cfo, 128 * cfi:128 * (cfi + 1)], ident[:])
                nc.scalar.copy(WT3[:, cfi, :], pt[:])

            # featT cols: [c_fi*128 + 0..3] = feat per sample, [+4] = b vec
            featT = const.tile([128, 512], F32)
            ft = po_pool.tile([128, 512], F32, tag="po")
            for cfi in range(NF):
                nc.tensor.transpose(
                    ft[:, 128 * cfi:128 * (cfi + 1)],
                    featn[:, 128 * cfi:128 * (cfi + 1)], ident[:])
            nc.scalar.copy(featT[:], ft[:])

            lp = po_pool.tile([128, 512], F32, tag="po")
            for cfo in range(NF):
                for cfi in range(NF):
                    nc.tensor.matmul(
                        lp[:, 16 * cfo:16 * cfo + 4],
                        WT3[:, cfi, 128 * cfo:128 * (cfo + 1)],
                        featT[:, 128 * cfi:128 * cfi + 4],
                        start=(cfi == 0), stop=(cfi == NF - 1))

            sigb = const.tile([128, NF], F32)
            tau = const.tile([128, NF * 4], F32)
            tau3 = tau[:].rearrange("p (c b) -> p c b", b=4)
            normt = const.tile([128, NF * 4], F32)
            norm3 = normt[:].rearrange("p (c b) -> p c b", b=4)
            dg = const.tile([128, BC * NF * 128], y_dt)
            dg3 = dg[:].rearrange("p (i f) -> p i f", f=128)
            CPY = mybir.ActivationFunctionType.Copy
            for c in range(NF):
                nc.vector.tensor_scalar(
                    sigb[:, c:c + 1], featT[:, 128 * c + 4:128 * c + 5],
                    0.1, -3.0, MULT, ADD)
                nc.scalar.activation(
                    tau3[:, c, :], lp[:, 16 * c:16 * c + 4], SIG,
                    bias=sigb[:, c:c + 1], scale=0.1)
                nc.vector.tensor_scalar(
                    norm3[:, c, :], tau3[:, c, :], -1.0, 1.0, MULT, ADD)
                for bi in range(BC):
                    nc.scalar.activation(
                        dg3[:, NF * bi + c, :], ident[:], CPY,
                        scale=norm3[:, c, bi:bi + 1])

            # ---- main pipelined loop ----
            # Global phases g = 0..BC*NL4+1.  Phase g runs the input stage
            # (DMA in / PE transpose / DVE scan) for sample g//4 chunk g%4,
            # plus the output stage (PE diag-matmul / ACT copy / DMA out)
            # for (s_out, tg) with 4*s_out+tg == g-2: output group tg needs
            # scan chunks <= tg+1, which finished in phase 4*s_out+tg+1, so
            # lag 2 gives one phase of slack while keeping the tail at two
            # output-only phases.
            ys_all = {}

            def emit_input_phase(bi, lc4, ys):
                xn = xn_pool.tile([128, 4 * 512], F32, tag="xn")
                xn3 = xn[:].rearrange("p (s f) -> p s f", f=512)
                nc.sync.dma_start(
                    xn3,
                    x_in[bi, 512 * lc4:512 * (lc4 + 1), :]
                    .rearrange("(s l) f -> l s f", l=128))
                for c in range(NF):
                    tb, yt = ys[c]
                    px = px_pool.tile([128, 512], F32, tag="px")
                    for s in range(4):
                        nc.tensor.transpose(
                            px[:, 128 * s:128 * (s + 1)],
                            xn3[:, s, 128 * c:128 * (c + 1)], ident[:])
                    init = 0.0 if lc4 == 0 else yt[:, 512 * lc4 - 1:512 * lc4]
                    nc.vector.tensor_tensor_scan(
                        yt[:, 512 * lc4:512 * (lc4 + 1)],
                        tb[:], px[:], init, MULT, ADD)

            def emit_output_phase(bi, tg, ys):
                # outputs: t-chunk k covers out rows [128k, 128k+128);
                # the final chunk overlaps (rows 1793..1920, rewritten
                # with identical values) so every matmul is full-width.
                st = st_pool.tile([128, 4 * 512], F32, tag="st")
                st3 = st[:].rearrange("p (j f) -> p j f", f=512)
                for j in range(4):
                    k = 4 * tg + j
                    t0 = (127 + 128 * k) if k < 15 else (L - 128)
                    po = po_pool.tile([128, 512], F32, tag="po")
                    for c in range(NF):
                        nc.tensor.matmul(
                            po[:, 128 * c:128 * (c + 1)],
                            ys[c][1][:, t0:t0 + 128],
                            dg3[:, NF * bi + c, :],
                            start=True, stop=True)
                    nc.scalar.copy(st3[:, j, :], po[:])
                if tg < 3:
                    nc.scalar.dma_start(
                        out_t[bi, 512 * tg:512 * (tg + 1), :]
                        .rearrange("(j t) f -> t j f", t=128), st3)
                else:
                    nc.scalar.dma_start(
                        out_t[bi, 1536:1920, :]
                        .rearrange("(j t) f -> t j f", t=128),
                        st3[:, 0:3, :])
                    # k=15 block covers rows 1793..1920; only row 1920
                    # (partition 127) is new — rows up to 1919 were
                    # written by the k=14 block.
                    nc.scalar.dma_start(out_t[bi, LOUT - 1:LOUT, :],
                                        st3[127:128, 3, :])

            for g in range(BC * NL4 + 2):
                s_in, ph_in = divmod(g, NL4)
                if s_in < BC and ph_in == 0:
                    ys = []
                    for c in range(NF):
                        tb = taub_pool.tile([128, 512], F32, tag="taub")
                        nc.scalar.activation(
                            tb[:], ones[:],
                            mybir.ActivationFunctionType.Copy,
                            scale=tau3[:, c, s_in:s_in + 1])
                        yt = y_pool.tile([128, L], y_dt, tag="y")
                        ys.append((tb, yt))
                    ys_all[s_in] = ys
                if g >= 2:
                    s_out, tg = divmod(g - 2, NL4)
                    emit_output_phase(s_out, tg, ys_all[s_out])
                if s_in < BC:
                    emit_input_phase(s_in, ph_in, ys_all[s_in])
    nc.compile()
    return nc


def _get_nc():
    if "nc" not in _CACHE:
        _CACHE["nc"] = _build()
    return _CACHE["nc"]


def kernel(x: np.ndarray, W: np.ndarray, b: np.ndarray) -> np.ndarray:
    from concourse.bass_utils import run_bass_kernel_spmd

    x = np.ascontiguousarray(x, dtype=np.float32)
    W = np.ascontiguousarray(W, dtype=np.float32)
    b = np.ascontiguousarray(b, dtype=np.float32)
    nc = _get_nc()
    in_maps = [
        {"x": x[i * BC:(i + 1) * BC], "W": W, "b": b} for i in range(NCORES)
    ]
    res = run_bass_kernel_spmd(nc, in_maps, list(range(NCORES)))
    return np.concatenate(
        [res.results[i]["out"] for i in range(NCORES)], axis=0)


if __name__ == "__main__":
    rng = np.random.default_rng(0)
    x = rng.standard_normal((B, L, F), dtype=np.float32)
    W = (rng.standard_normal((F, F), dtype=np.float32) / np.sqrt(F)).astype(np.float32)
    b = np.zeros((F,), dtype=np.float32)
    out = kernel(x, W, b)
    print("out", out.shape, out.dtype)


# revision 11
# speedup vs baseline: 1.0244x; 1.0244x over previous
"""Trainium2 Bass kernel for nn_ConvFilter (dense_cnn).

Math: tau = sigmoid((x[:,-1,:] @ W.T + b)/10 - 3) in (0, 0.08];
norm = 1/sum_i tau^i = (1-tau)/(1-tau^128) == 1-tau in fp32 (tau^128
underflows); out[b,t,f] = norm * sum_{j=0}^{127} tau^j x[b, t+127-j, f]
== norm * y[b, t+127, f] where y is the infinite exponential filter
y[s] = tau*y[s-1] + x[s] (extra terms carry tau^{>=128} == 0 in fp32).
y is computed with the DVE tensor_tensor_scan recurrence.

Sharding: data-parallel over batch B=32 across 8 cores (4 samples each);
W, b replicated.  Per core: PE transposes x[l,f] tiles into [f,l] PSUM
banks, DVE scans them (chained, state carried across 512-wide chunks),
PE applies norm + transposes back via diag(norm) matmuls, ACT copies
PSUM->SBUF staging, DMA out.

v2: software-pipelined emission — the output stage of sample bi-1
(PE diag-matmuls + ACT copies + DMA out) is interleaved chunk-by-chunk
with the input stage of sample bi (DMA in + PE transposes + DVE scans)
so every engine's in-order queue always has ready work.  y and the
diag(norm) matrices are fp16 (scan state stays fp32 internally), which
runs the 256 output matmuls/core at 1 cycle/row instead of fp32's 4.
"""
import os

import numpy as np

B, L, F, K = 32, 2048, 512, 128
LOUT = L - K + 1  # 1921
NCORES = 8
BC = B // NCORES  # 4 batch samples per core
NF = F // 128     # 4 feature chunks
NL4 = 4           # 1 MiB input chunks per sample (each 512 time steps)

_CACHE = {}


def _build():
    import concourse.bacc as bacc
    import concourse.tile as tile
    from concourse import masks, mybir

    F32 = mybir.dt.float32
    F16 = mybir.dt.float16
    MULT = mybir.AluOpType.mult
    ADD = mybir.AluOpType.add
    SIG = mybir.ActivationFunctionType.Sigmoid

    y_dt = F16 if os.environ.get("Y_DT", "f16") == "f16" else F32

    nc = bacc.Bacc("TRN2", target_bir_lowering=False, debug=False,
                   num_devices=NCORES)
    x_in = nc.dram_tensor("x", [BC, L, F], F32, kind="ExternalInput")
    w_in = nc.dram_tensor("W", [F, F], F32, kind="ExternalInput")
    b_in = nc.dram_tensor("b", [F], F32, kind="ExternalInput")
    out_t = nc.dram_tensor("out", [BC, LOUT, F], F32, kind="ExternalOutput")

    with tile.TileContext(nc) as tc:
        with (
            tc.tile_pool(name="const", bufs=1) as const,
            tc.tile_pool(name="xn", bufs=8) as xn_pool,
            tc.tile_pool(name="taub", bufs=8) as taub_pool,
            tc.tile_pool(name="y", bufs=8) as y_pool,
            tc.tile_pool(name="st", bufs=4) as st_pool,
            tc.tile_pool(name="px", bufs=4, space="PSUM") as px_pool,
            tc.tile_pool(name="po", bufs=3, space="PSUM") as po_pool,
        ):
            # ---- phase 0: tau / norm / diag(norm) ----
            # tau-phase loads go on the ACT HWDGE ring so the Sync ring is
            # free for the (dependency-less) x input stream from t=0.  The
            # W load is first (nothing may queue ahead of it on the ring —
            # PE's whole tau phase waits on W) and is split by fi-chunk so
            # the transposes pipeline with the remaining chunks' DMA.
            wn = const.tile([128, NF * 512], F32)
            wn3 = wn[:].rearrange("p (c f) -> p c f", f=512)
            for cfi in range(NF):
                nc.scalar.dma_start(
                    wn3[:, :, 128 * cfi:128 * (cfi + 1)],
                    w_in[:, 128 * cfi:128 * (cfi + 1)]
                    .rearrange("(c p) f -> p c f", p=128))
            # rows 0-3 feat, row 4 = b; rows 5.. stay garbage — they only
            # reach featT columns that no consumer reads.
            featn = const.tile([128, 512], F32)
            nc.scalar.dma_start(featn[0:BC, :], x_in[:, L - 1, :])
            nc.scalar.dma_start(featn[4:5, :], b_in[None, :])

            ident = const.tile([128, 128], F32)
            masks.make_identity(nc, ident[:])
            ones = const.tile([128, 512], F32)
            nc.gpsimd.memset(ones[:], 1.0)

            WT = const.tile([128, NF * 512], F32)   # [fi, (c_fi, fo)]
            WT3 = WT[:].rearrange("p (c f) -> p c f", f=512)
            for cfi in range(NF):
                pt = po_pool.tile([128, 512], F32, tag="po")
                for cfo in range(NF):
                    nc.tensor.transpose(
                        pt[:, 128 * cfo:128 * (cfo + 1)],
                        wn3[:, cfo, 128 * cfi:128 * (cfi + 1)], ident[:])
                nc.scalar.copy(WT3[:, cfi, :], pt[:])

            # featT cols: [c_fi*128 + 0..3] = feat per sample, [+4] = b vec
            featT = const.tile([128, 512], F32)
            ft = po_pool.tile([128, 512], F32, tag="po")
            for cfi in range(NF):
                nc.tensor.transpose(
                    ft[:, 128 * cfi:128 * (cfi + 1)],
                    featn[:, 128 * cfi:128 * (cfi + 1)], ident[:])
            nc.scalar.copy(featT[:], ft[:])

            lp = po_pool.tile([128, 512], F32, tag="po")
            for cfo in range(NF):
                for cfi in range(NF):
                    nc.tensor.matmul(
                        lp[:, 16 * cfo:16 * cfo + 4],
                        WT3[:, cfi, 128 * cfo:128 * (cfo + 1)],
                        featT[:, 128 * cfi:128 * cfi + 4],
                        start=(cfi == 0), stop=(cfi == NF - 1))

            sigb = const.tile([128, NF], F32)
            tau = const.tile([128, NF * 4], F32)
            tau3 = tau[:].rearrange("p (c b) -> p c b", b=4)
            normt = const.tile([128, NF * 4], F32)
            norm3 = normt[:].rearrange("p (c b) -> p c b", b=4)
            dg = const.tile([128, BC * NF * 128], y_dt)
            dg3 = dg[:].rearrange("p (i f) -> p i f", f=128)
            CPY = mybir.ActivationFunctionType.Copy
            for c in range(NF):
                nc.vector.tensor_scalar(
                    sigb[:, c:c + 1], featT[:, 128 * c + 4:128 * c + 5],
                    0.1, -3.0, MULT, ADD)
                nc.scalar.activation(
                    tau3[:, c, :], lp[:, 16 * c:16 * c + 4], SIG,
                    bias=sigb[:, c:c + 1], scale=0.1)
                nc.vector.tensor_scalar(
                    norm3[:, c, :], tau3[:, c, :], -1.0, 1.0, MULT, ADD)
                for bi in range(BC):
                    nc.scalar.activation(
                        dg3[:, NF * bi + c, :], ident[:], CPY,
                        scale=norm3[:, c, bi:bi + 1])

            # ---- main pipelined loop ----
            # Global phases g = 0..BC*NL4+1.  Phase g runs the input stage
            # (DMA in / PE transpose / DVE scan) for sample g//4 chunk g%4,
            # plus the output stage (PE diag-matmul / ACT copy / DMA out)
            # for (s_out, tg) with 4*s_out+tg == g-2: output group tg needs
            # scan chunks <= tg+1, which finished in phase 4*s_out+tg+1, so
            # lag 2 gives one phase of slack while keeping the tail at two
            # output-only phases.
            ys_all = {}

            def emit_input_phase(bi, lc4, ys):
                xn = xn_pool.tile([128, 4 * 512], F32, tag="xn")
                xn3 = xn[:].rearrange("p (s f) -> p s f", f=512)
                nc.sync.dma_start(
                    xn3,
                    x_in[bi, 512 * lc4:512 * (lc4 + 1), :]
                    .rearrange("(s l) f -> l s f", l=128))
                for c in range(NF):
                    tb, yt = ys[c]
                    px = px_pool.tile([128, 512], F32, tag="px")
                    for s in range(4):
                        nc.tensor.transpose(
                            px[:, 128 * s:128 * (s + 1)],
                            xn3[:, s, 128 * c:128 * (c + 1)], ident[:])
                    init = 0.0 if lc4 == 0 else yt[:, 512 * lc4 - 1:512 * lc4]
                    nc.vector.tensor_tensor_scan(
                        yt[:, 512 * lc4:512 * (lc4 + 1)],
                        tb[:], px[:], init, MULT, ADD)

            def emit_output_phase(bi, tg, ys):
                # outputs: t-chunk k covers out rows [128k, 128k+128);
                # the final chunk overlaps (rows 1793..1920, rewritten
                # with identical values) so every matmul is full-width.
                st = st_pool.tile([128, 4 * 512], F32, tag="st")
                st3 = st[:].rearrange("p (j f) -> p j f", f=512)
                for j in range(4):
                    k = 4 * tg + j
                    t0 = (127 + 128 * k) if k < 15 else (L - 128)
                    po = po_pool.tile([128, 512], F32, tag="po")
                    for c in range(NF):
                        nc.tensor.matmul(
                            po[:, 128 * c:128 * (c + 1)],
                            ys[c][1][:, t0:t0 + 128],
                            dg3[:, NF * bi + c, :],
                            start=True, stop=True)
                    nc.scalar.copy(st3[:, j, :], po[:])
                if tg < 3:
                    nc.scalar.dma_start(
                        out_t[bi, 512 * tg:512 * (tg + 1), :]
                        .rearrange("(j t) f -> t j f", t=128), st3)
                else:
                    nc.scalar.dma_start(
                        out_t[bi, 1536:1920, :]
                        .rearrange("(j t) f -> t j f", t=128),
                        st3[:, 0:3, :])
                    # k=15 block covers rows 1793..1920; only row 1920
                    # (partition 127) is new — rows up to 1919 were
                    # written by the k=14 block.
                    nc.scalar.dma_start(out_t[bi, LOUT - 1:LOUT, :],
                                        st3[127:128, 3, :])

            for g in range(BC * NL4 + 2):
                s_in, ph_in = divmod(g, NL4)
                if s_in < BC and ph_in == 0:
                    ys = []
                    for c in range(NF):
                        tb = taub_pool.tile([128, 512], F32, tag="taub")
                        nc.scalar.activation(
                            tb[:], ones[:],
                            mybir.ActivationFunctionType.Copy,
                            scale=tau3[:, c, s_in:s_in + 1])
                        yt = y_pool.tile([128, L], y_dt, tag="y")
                        ys.append((tb, yt))
                    ys_all[s_in] = ys
                if g >= 2:
                    s_out, tg = divmod(g - 2, NL4)
                    emit_output_phase(s_out, tg, ys_all[s_out])
                if s_in < BC:
                    emit_input_phase(s_in, ph_in, ys_all[s_in])
    nc.compile()
    return nc


def _get_nc():
    if "nc" not in _CACHE:
        _CACHE["nc"] = _build()
    return _CACHE["nc"]


def kernel(x: np.ndarray, W: np.ndarray, b: np.ndarray) -> np.ndarray:
    from concourse.bass_utils import run_bass_kernel_spmd

    x = np.ascontiguousarray(x, dtype=np.float32)
    W = np.ascontiguousarray(W, dtype=np.float32)
    b = np.ascontiguousarray(b, dtype=np.float32)
    nc = _get_nc()
    in_maps = [
        {"x": x[i * BC:(i + 1) * BC], "W": W, "b": b} for i in range(NCORES)
    ]
    res = run_bass_kernel_spmd(nc, in_maps, list(range(NCORES)))
    return np.concatenate(
        [res.results[i]["out"] for i in range(NCORES)], axis=0)


if __name__ == "__main__":
    rng = np.random.default_rng(0)
    x = rng.standard_normal((B, L, F), dtype=np.float32)
    W = (rng.standard_normal((F, F), dtype=np.float32) / np.sqrt(F)).astype(np.float32)
    b = np.zeros((F,), dtype=np.float32)
    out = kernel(x, W, b)
    print("out", out.shape, out.dtype)


# revision 15
# speedup vs baseline: 1.0295x; 1.0050x over previous
"""Trainium2 Bass kernel for nn_ConvFilter (dense_cnn).

Math: tau = sigmoid((x[:,-1,:] @ W.T + b)/10 - 3) in (0, 0.08];
norm = 1/sum_i tau^i = (1-tau)/(1-tau^128) == 1-tau in fp32 (tau^128
underflows); out[b,t,f] = norm * sum_{j=0}^{127} tau^j x[b, t+127-j, f]
== norm * y[b, t+127, f] where y is the infinite exponential filter
y[s] = tau*y[s-1] + x[s] (extra terms carry tau^{>=128} == 0 in fp32).
y is computed with the DVE tensor_tensor_scan recurrence.

Sharding: data-parallel over batch B=32 across 8 cores (4 samples each);
W, b replicated.  Per core: PE transposes x[l,f] tiles into [f,l] PSUM
banks, DVE scans them (chained, state carried across 512-wide chunks),
PE applies norm + transposes back via diag(norm) matmuls, ACT copies
PSUM->SBUF staging, DMA out.

v2: software-pipelined emission — the output stage of sample bi-1
(PE diag-matmuls + ACT copies + DMA out) is interleaved chunk-by-chunk
with the input stage of sample bi (DMA in + PE transposes + DVE scans)
so every engine's in-order queue always has ready work.  y and the
diag(norm) matrices are fp16 (scan state stays fp32 internally), which
runs the 256 output matmuls/core at 1 cycle/row instead of fp32's 4.
"""
import os

import numpy as np

B, L, F, K = 32, 2048, 512, 128
LOUT = L - K + 1  # 1921
NCORES = 8
BC = B // NCORES  # 4 batch samples per core
NF = F // 128     # 4 feature chunks
NL4 = 4           # 1 MiB input chunks per sample (each 512 time steps)

_CACHE = {}


def _build():
    import concourse.bacc as bacc
    import concourse.tile as tile
    from concourse import masks, mybir

    F32 = mybir.dt.float32
    F16 = mybir.dt.float16
    MULT = mybir.AluOpType.mult
    ADD = mybir.AluOpType.add
    SIG = mybir.ActivationFunctionType.Sigmoid

    y_dt = F16 if os.environ.get("Y_DT", "f16") == "f16" else F32

    nc = bacc.Bacc("TRN2", target_bir_lowering=False, debug=False,
                   num_devices=NCORES)
    x_in = nc.dram_tensor("x", [BC, L, F], F32, kind="ExternalInput")
    w_in = nc.dram_tensor("W", [F, F], F32, kind="ExternalInput")
    b_in = nc.dram_tensor("b", [F], F32, kind="ExternalInput")
    out_t = nc.dram_tensor("out", [BC, LOUT, F], F32, kind="ExternalOutput")

    with tile.TileContext(nc) as tc:
        with (
            tc.tile_pool(name="const", bufs=1) as const,
            tc.tile_pool(name="xn", bufs=8) as xn_pool,
            tc.tile_pool(name="taub", bufs=8) as taub_pool,
            tc.tile_pool(name="y", bufs=8) as y_pool,
            tc.tile_pool(name="st", bufs=4) as st_pool,
            tc.tile_pool(name="px", bufs=4, space="PSUM") as px_pool,
            tc.tile_pool(name="po", bufs=3, space="PSUM") as po_pool,
        ):
            # ---- phase 0: tau / norm / diag(norm) ----
            # Minimize latency to the first scan: everything the first scan
            # transitively needs (W -> W^T -> logits -> sigmoid -> tau
            # broadcast) is ordered first on each engine; deferrable work
            # (dg fills, later samples' broadcasts) comes after.
            CPY = mybir.ActivationFunctionType.Copy
            # Dummy activation: pull the sigmoid ACT table load off the
            # critical path (it would otherwise load lazily at first use).
            scr = const.tile([2, 8], F32)
            nc.scalar.activation(scr[:], scr[:], SIG)

            # tau-phase loads go on the ACT HWDGE ring so the Sync ring is
            # free for the (dependency-less) x input stream from t=0.
            featn = const.tile([128, 512], F32)
            # rows 0-3 feat, row 4 = b; rows 5.. stay garbage — they only
            # reach featT columns that no consumer reads.
            nc.scalar.dma_start(featn[0:BC, :], x_in[:, L - 1, :])
            nc.scalar.dma_start(featn[4:5, :], b_in[None, :])
            wn = const.tile([128, NF * 512], F32)
            wn3 = wn[:].rearrange("p (c f) -> p c f", f=512)
            nc.scalar.dma_start(wn3, w_in[:, :].rearrange("(c p) f -> p c f", p=128))

            ident = const.tile([128, 128], F32)
            masks.make_identity(nc, ident[:])
            ones = const.tile([128, 512], F32)
            nc.gpsimd.memset(ones[:], 1.0)

            # featT cols: [c_fi*128 + 0..3] = feat per sample, [+4] = b vec
            featT = const.tile([128, 512], F32)
            ft = po_pool.tile([128, 512], F32, tag="po")
            for cfi in range(NF):
                nc.tensor.transpose(
                    ft[:, 128 * cfi:128 * (cfi + 1)],
                    featn[:, 128 * cfi:128 * (cfi + 1)], ident[:])
            nc.scalar.copy(featT[:], ft[:])

            # W^T chunk cfi, then immediately the 4 logit matmuls that
            # consume it (PSUM-accumulated across cfi) so lp trails the
            # last transpose by one group instead of the whole phase.
            WT = const.tile([128, NF * 512], F32)   # [fi, (c_fi, fo)]
            WT3 = WT[:].rearrange("p (c f) -> p c f", f=512)
            lp = po_pool.tile([128, 512], F32, tag="po")
            for cfi in range(NF):
                pt = po_pool.tile([128, 512], F32, tag="po")
                for cfo in range(NF):
                    nc.tensor.transpose(
                        pt[:, 128 * cfo:128 * (cfo + 1)],
                        wn3[:, cfo, 128 * cfi:128 * (cfi + 1)], ident[:])
                nc.scalar.copy(WT3[:, cfi, :], pt[:])
                for cfo in range(NF):
                    nc.tensor.matmul(
                        lp[:, 16 * cfo:16 * cfo + 4],
                        WT3[:, cfi, 128 * cfo:128 * (cfo + 1)],
                        featT[:, 128 * cfi:128 * cfi + 4],
                        start=(cfi == 0), stop=(cfi == NF - 1))

            sigb = const.tile([128, NF], F32)
            tau = const.tile([128, NF * 4], F32)
            tau3 = tau[:].rearrange("p (c b) -> p c b", b=4)
            normt = const.tile([128, NF * 4], F32)
            norm3 = normt[:].rearrange("p (c b) -> p c b", b=4)
            dg = const.tile([128, BC * NF * 128], y_dt)
            dg3 = dg[:].rearrange("p (i f) -> p i f", f=128)
            tb0 = []
            for c in range(NF):
                nc.vector.tensor_scalar(
                    sigb[:, c:c + 1], featT[:, 128 * c + 4:128 * c + 5],
                    0.1, -3.0, MULT, ADD)
                nc.scalar.activation(
                    tau3[:, c, :], lp[:, 16 * c:16 * c + 4], SIG,
                    bias=sigb[:, c:c + 1], scale=0.1)
                # sample 0's tau broadcast right behind its sigmoid — the
                # first scans need it; everything else can wait.
                tb = taub_pool.tile([128, 512], F32, tag="taub")
                nc.scalar.activation(tb[:], ones[:], CPY,
                                     scale=tau3[:, c, 0:1])
                tb0.append(tb)
                nc.vector.tensor_scalar(
                    norm3[:, c, :], tau3[:, c, :], -1.0, 1.0, MULT, ADD)
            for c in range(NF):
                for bi in range(BC):
                    nc.scalar.activation(
                        dg3[:, NF * bi + c, :], ident[:], CPY,
                        scale=norm3[:, c, bi:bi + 1])

            # ---- main pipelined loop ----
            # Global phases g = 0..BC*NL4+1.  Phase g runs the input stage
            # (DMA in / PE transpose / DVE scan) for sample g//4 chunk g%4,
            # plus the output stage (PE diag-matmul / ACT copy / DMA out)
            # for (s_out, tg) with 4*s_out+tg == g-2: output group tg needs
            # scan chunks <= tg+1, which finished in phase 4*s_out+tg+1, so
            # lag 2 gives one phase of slack while keeping the tail at two
            # output-only phases.
            ys_all = {}

            def emit_input_phase(bi, lc4, ys):
                xn = xn_pool.tile([128, 4 * 512], F32, tag="xn")
                xn3 = xn[:].rearrange("p (s f) -> p s f", f=512)
                nc.sync.dma_start(
                    xn3,
                    x_in[bi, 512 * lc4:512 * (lc4 + 1), :]
                    .rearrange("(s l) f -> l s f", l=128))
                for c in range(NF):
                    tb, yt = ys[c]
                    px = px_pool.tile([128, 512], F32, tag="px")
                    for s in range(4):
                        nc.tensor.transpose(
                            px[:, 128 * s:128 * (s + 1)],
                            xn3[:, s, 128 * c:128 * (c + 1)], ident[:])
                    init = 0.0 if lc4 == 0 else yt[:, 512 * lc4 - 1:512 * lc4]
                    nc.vector.tensor_tensor_scan(
                        yt[:, 512 * lc4:512 * (lc4 + 1)],
                        tb[:], px[:], init, MULT, ADD)

            def emit_output_phase(bi, tg, ys):
                # outputs: t-chunk k covers out rows [128k, 128k+128);
                # the final chunk overlaps (rows 1793..1920, rewritten
                # with identical values) so every matmul is full-width.
                # The last sample's groups split their store DMA in two so
                # the pipeline drain isn't gated on all four copies.
                split = bi == BC - 1
                st = st_pool.tile([128, 4 * 512], F32, tag="st")
                st3 = st[:].rearrange("p (j f) -> p j f", f=512)
                for j in range(4):
                    k = 4 * tg + j
                    t0 = (127 + 128 * k) if k < 15 else (L - 128)
                    po = po_pool.tile([128, 512], F32, tag="po")
                    for c in range(NF):
                        nc.tensor.matmul(
                            po[:, 128 * c:128 * (c + 1)],
                            ys[c][1][:, t0:t0 + 128],
                            dg3[:, NF * bi + c, :],
                            start=True, stop=True)
                    nc.scalar.copy(st3[:, j, :], po[:])
                    if split and j == 1:
                        nc.scalar.dma_start(
                            out_t[bi, 512 * tg:512 * tg + 256, :]
                            .rearrange("(j t) f -> t j f", t=128),
                            st3[:, 0:2, :])
                if tg < 3:
                    if split:
                        nc.scalar.dma_start(
                            out_t[bi, 512 * tg + 256:512 * (tg + 1), :]
                            .rearrange("(j t) f -> t j f", t=128),
                            st3[:, 2:4, :])
                    else:
                        nc.scalar.dma_start(
                            out_t[bi, 512 * tg:512 * (tg + 1), :]
                            .rearrange("(j t) f -> t j f", t=128), st3)
                else:
                    if split:
                        nc.scalar.dma_start(
                            out_t[bi, 1792:1920, :], st3[:, 2, :])
                    else:
                        nc.scalar.dma_start(
                            out_t[bi, 1536:1920, :]
                            .rearrange("(j t) f -> t j f", t=128),
                            st3[:, 0:3, :])
                    # k=15 block covers rows 1793..1920; only row 1920
                    # (partition 127) is new — rows up to 1919 were
                    # written by the k=14 block.
                    nc.scalar.dma_start(out_t[bi, LOUT - 1:LOUT, :],
                                        st3[127:128, 3, :])

            for g in range(BC * NL4 + 2):
                s_in, ph_in = divmod(g, NL4)
                if s_in < BC and ph_in == 0:
                    ys = []
                    for c in range(NF):
                        if s_in == 0:
                            tb = tb0[c]
                        else:
                            tb = taub_pool.tile([128, 512], F32, tag="taub")
                            nc.scalar.activation(
                                tb[:], ones[:],
                                mybir.ActivationFunctionType.Copy,
                                scale=tau3[:, c, s_in:s_in + 1])
                        yt = y_pool.tile([128, L], y_dt, tag="y")
                        ys.append((tb, yt))
                    ys_all[s_in] = ys
                if g >= 2:
                    s_out, tg = divmod(g - 2, NL4)
                    emit_output_phase(s_out, tg, ys_all[s_out])
                if s_in < BC:
                    emit_input_phase(s_in, ph_in, ys_all[s_in])
    nc.compile()
    return nc


def _get_nc():
    if "nc" not in _CACHE:
        _CACHE["nc"] = _build()
    return _CACHE["nc"]


def kernel(x: np.ndarray, W: np.ndarray, b: np.ndarray) -> np.ndarray:
    from concourse.bass_utils import run_bass_kernel_spmd

    x = np.ascontiguousarray(x, dtype=np.float32)
    W = np.ascontiguousarray(W, dtype=np.float32)
    b = np.ascontiguousarray(b, dtype=np.float32)
    nc = _get_nc()
    in_maps = [
        {"x": x[i * BC:(i + 1) * BC], "W": W, "b": b} for i in range(NCORES)
    ]
    res = run_bass_kernel_spmd(nc, in_maps, list(range(NCORES)))
    return np.concatenate(
        [res.results[i]["out"] for i in range(NCORES)], axis=0)


if __name__ == "__main__":
    rng = np.random.default_rng(0)
    x = rng.standard_normal((B, L, F), dtype=np.float32)
    W = (rng.standard_normal((F, F), dtype=np.float32) / np.sqrt(F)).astype(np.float32)
    b = np.zeros((F,), dtype=np.float32)
    out = kernel(x, W, b)
    print("out", out.shape, out.dtype)
